# revision 25
# baseline (speedup 1.0000x reference)
"""GCN (2-layer, PyG gcn_norm) on 8 Trainium2 NeuronCores via Bass.

Strategy (dst-partition-row sharding, no collectives, no PE):
  * Host appends self-loop edges (weight 1, as in gcn_norm), sorts nodes
    by in-degree and assigns each node one SBUF partition-row of G slots
    (G = per-stripe max degree rounded up to 8; ~12% padding), so the
    per-node segment-sum needs no one-hot masks or matmuls.  Stripes of
    1024 nodes (one 128-node tile per core) share a G schedule so all 8
    SPMD cores run one program.
  * Per-edge streams are bf16 (tolerance is 2e-2).  The segment-sum runs
    as a packed-bf16 pair-add tree on DVE (tensor_tensor at the 2x rate;
    TensorReduce itself has no fast mode) with a final narrow f32
    tensor_reduce per equal-G run.  The layer-1 node epilogue
    (z->W1->relu->W2) uses weight tiles pre-materialized in (hidden,
    tile) layout so every product is a packed 2x tensor_tensor.
  * Streams transfer as whole contiguous blocks split across the two
    hardware DGE queues (SP + Activation); the Activation engine is kept
    compute-free so its queue can prefetch ahead of the epilogue.
  * Three sequential NEFF launches: (1) deg -> dinv, x*dinv, (2) layer-1
    aggregation -> h -> v*dinv, (3) layer-2 aggregation -> output.
    Between launches the host only gathers returned per-node arrays into
    per-edge streams (index-space data movement, no float math).
"""

import sys

sys.path.insert(0, "/opt/trn_rl_repo")

import numpy as np
import ml_dtypes

import concourse.bass as bass
import concourse.tile as tile
from concourse import mybir
from concourse.bass_utils import run_bass_kernel_spmd

BF16 = ml_dtypes.bfloat16

N = 100000
E = 3200000
D = 2
HID = 16
NCORE = 8
TPC = 98                      # stripes == node tiles per core
NPAD = TPC * 1024             # 100352
GMULT = 8                     # stripe slot width rounded up to this
BLK_COLS = 4096               # target stream columns per DMA block (>= CS: single block)


def _split_multi_waits(nc):
    """This toolchain's walrus encodes at most one sync-wait per instruction.
    Hoist extra waits onto fresh single-wait NoOps placed just before."""
    ctr = 0
    for fn in nc.m.functions:
        for bb in fn.blocks:
            insts = list(bb.instructions)
            if not any(
                i.sync_info is not None and len(i.sync_info.on_wait or []) > 1
                for i in insts
            ):
                continue
            new = []
            for inst in insts:
                si = inst.sync_info
                if si is not None and len(si.on_wait or []) > 1:
                    waits = list(si.on_wait)
                    for w in waits[:-1]:
                        ctr += 1
                        new.append(
                            mybir.InstNoOp(
                                name=f"wsplit-{ctr}",
                                engine=inst.engine,
                                sync_info=mybir.SyncInfo(on_wait=[w], on_update=[]),
                                bass_nofuse=True,
                            )
                        )
                    si.on_wait = [waits[-1]]
                new.append(inst)
            bb.instructions = new
    return ctr


def _preprocess(edge_index, edge_weight):
    """Append self-loops, degree-sort nodes, assign each node a
    partition-row slot range, and scatter edge weight / src index into the
    per-core slot streams."""
    loop = np.arange(N, dtype=np.int64)
    dst = np.concatenate([edge_index[1].astype(np.int64), loop])
    src = np.concatenate([edge_index[0].astype(np.int64), loop])
    ew = np.concatenate([edge_weight.astype(np.float32),
                         np.ones(N, np.float32)])
    ne = len(dst)

    deg = np.bincount(dst, minlength=NPAD)
    order = np.argsort(deg, kind="stable")       # newpos -> orig id
    newpos = np.empty(NPAD, np.int64)
    newpos[order] = np.arange(NPAD)

    counts_new = deg[order]                      # per-newpos degree
    smax = counts_new.reshape(TPC, 1024).max(axis=1)
    G = np.maximum(GMULT, ((smax + GMULT - 1) // GMULT) * GMULT).astype(np.int64)
    offs = np.zeros(TPC + 1, np.int64)
    np.cumsum(G, out=offs[1:])
    CS = int(offs[-1])

    nd = newpos[dst]
    start = np.zeros(NPAD + 1, np.int64)
    np.cumsum(counts_new, out=start[1:])
    perm = np.argsort(nd, kind="stable")
    r = np.empty(ne, np.int64)
    r[perm] = np.arange(ne) - start[nd[perm]]    # rank of edge within its dst

    s = nd >> 10
    w = nd & 1023
    c = w >> 7
    p = w & 127
    flat = (c * 128 + p) * CS + offs[s] + r

    ew_flat = np.zeros(NCORE * 128 * CS, np.float32)
    src_flat = np.zeros(NCORE * 128 * CS, np.int64)
    ew_flat[flat] = ew
    src_flat[flat] = src

    # DMA blocks: consecutive stripes until >= BLK_COLS columns; per-block
    # runs of stripes sharing G (one tensor_reduce instruction per run).
    blocks = []
    t0, cols = 0, 0
    for t in range(TPC):
        cols += int(G[t])
        if cols >= BLK_COLS or t == TPC - 1:
            runs = []
            ro = 0
            for tt in range(t0, t + 1):
                g = int(G[tt])
                if runs and runs[-1][2] == g:
                    runs[-1] = (runs[-1][0], runs[-1][1] + 1, g, runs[-1][3])
                else:
                    runs.append((tt, 1, g, ro))
                ro += g
            blocks.append((t0, t + 1 - t0, int(offs[t0]), cols, runs))
            t0, cols = t + 1, 0

    return dict(G=G, offs=offs, CS=CS, blocks=blocks, order=order,
                ew=ew_flat, src=src_flat)


def _stream_blocks(sched, arrflat, prefix, dtype):
    """Per-core dicts of per-DMA-block contiguous stream arrays."""
    CS = sched["CS"]
    a = arrflat.reshape(NCORE, 128, CS)
    out = []
    for c in range(NCORE):
        d = {}
        for bi, (t0, ntb, c0, bc, runs) in enumerate(sched["blocks"]):
            d[f"{prefix}{bi}"] = np.ascontiguousarray(
                a[c, :, c0:c0 + bc]).astype(dtype)
        out.append(d)
    return out


def _to_core_nodes(val_new, dtype):
    """[NPAD] array in newpos space -> per-core [128, TPC]
    (newpos = s*1024 + c*128 + p)."""
    a = val_new.reshape(TPC, NCORE, 128)
    return [np.ascontiguousarray(a[:, c, :].T).astype(dtype) for c in range(NCORE)]


def _from_core_nodes(parts):
    full = np.empty((TPC, NCORE, 128), np.float32)
    for c in range(NCORE):
        full[:, c, :] = np.asarray(parts[c], np.float32).T
    return full.reshape(NPAD)


def _build_sweep(mode, sched, reps=1, variant=None, unroll=16,
                 skip_b1=True, skip_b2=True, npos=HID):
    """Build the Bass program for one sweep. mode in {deg, layer1, layer2}.
    reps>1 wraps `reps` copies of the (idempotent) body in a hardware For_i
    loop, `unroll` bodies per trip — used only for timing measurements.
    variant (timing experiments only): 'dmaonly' = stream DMA without
    compute, 'reduceonly' = compute without stream DMA."""
    from contextlib import ExitStack

    CS = sched["CS"]
    blocks = sched["blocks"]
    BCMAX = max(b[3] for b in blocks)
    f32 = mybir.dt.float32
    bf = mybir.dt.bfloat16

    nc = bass.Bass("TRN2", target_bir_lowering=False, debug=False,
                   num_devices=NCORE)

    def din(name, shape, dtype=f32):
        return nc.dram_tensor(name, shape, dtype, kind="ExternalInput").ap()

    def dout(name, shape, dtype=f32):
        return nc.dram_tensor(name, shape, dtype, kind="ExternalOutput").ap()

    def din_blocks(prefix):
        return [din(f"{prefix}{bi}", [128, b[3]], bf)
                for bi, b in enumerate(blocks)]

    ew_d = din_blocks("ew")
    if mode == "deg":
        xn_d = [din(f"xn{f}", [128, TPC]) for f in range(D)]
        deg_out = dout("degout", [128, (D + 1) * TPC])
    elif mode == "layer1":
        xs_d = [din_blocks(f"xs{f}_") for f in range(D)]
        dinv_d = din("dinvn", [128, TPC])
        w1x_d = [din(f"w1x{f}b", [128, HID * TPC], bf) for f in range(D)]
        b1_d = din("b1b", [128, HID], bf)
        vt_out = dout("vtout", [128, TPC])
    else:
        vs_d = din_blocks("vs")
        dinv_d = din("dinvn", [128, TPC])
        b2_d = din("b2b", [128, 1])
        y_out = dout("yout", [128, TPC])

    with tile.TileContext(nc) as tc, ExitStack() as ctx:
        nbuf = 3 if mode == "layer1" else 6
        const = ctx.enter_context(tc.tile_pool(name="const", bufs=1))
        sp = ctx.enter_context(tc.tile_pool(name="streams", bufs=nbuf))
        wp = ctx.enter_context(tc.tile_pool(name="work", bufs=nbuf))
        accp = ctx.enter_context(tc.tile_pool(name="acc", bufs=2))

        if mode == "deg":
            xN = []
            for f in range(D):
                t_ = const.tile([128, TPC], f32, tag=f"xn{f}")
                nc.sync.dma_start(t_[:], xn_d[f][:])
                xN.append(t_)
        elif mode == "layer1":
            dinvN = const.tile([128, TPC], f32)
            nc.sync.dma_start(dinvN[:], dinv_d[:])
            w1x = []
            for f in range(D):
                t_ = const.tile([128, HID * TPC], bf, tag=f"w1x{f}")
                nc.sync.dma_start(t_[:], w1x_d[f][:])
                w1x.append(t_)
            b1_sb = const.tile([128, HID], bf)
            nc.sync.dma_start(b1_sb[:], b1_d[:])
        else:
            dinvN = const.tile([128, TPC], f32)
            nc.sync.dma_start(dinvN[:], dinv_d[:])
            b2_sb = const.tile([128, 1], f32)
            nc.sync.dma_start(b2_sb[:], b2_d[:])

        nF = D if mode == "layer1" else 1

        def _reduce_2stage(m_t, bc, runs, dst_agg, s1tag):
            # Pair-add tree: TensorReduce has no fast DVE mode (1x), but
            # TensorTensor adds on packed bf16 sub-slices run at 2x.  Two
            # tree levels collapse each 8-slot group to 2 partials; a final
            # narrow f32 TensorReduce finishes per G-run.
            u = wp.tile([128, BCMAX // 2], bf, tag=s1tag + "u")
            mv = m_t[:, 0:bc].rearrange("p (q g) -> p q g", g=8)
            nc.vector.tensor_tensor(
                u[:, 0:bc // 2].rearrange("p (q h) -> p q h", h=4),
                mv[:, :, 0:4], mv[:, :, 4:8], mybir.AluOpType.add)
            w = wp.tile([128, BCMAX // 4], bf, tag=s1tag + "w")
            uv = u[:, 0:bc // 2].rearrange("p (q h) -> p q h", h=4)
            nc.vector.tensor_tensor(
                w[:, 0:bc // 4].rearrange("p (q h) -> p q h", h=2),
                uv[:, :, 0:2], uv[:, :, 2:4], mybir.AluOpType.add)
            for (tt, nt, g, ro) in runs:
                nc.vector.tensor_reduce(
                    dst_agg[:, tt:tt + nt],
                    w[:, ro // 4:ro // 4 + nt * (g // 4)].rearrange(
                        "p (t q) -> p t q", q=g // 4),
                    mybir.AxisListType.X, mybir.AluOpType.add)

        def body():
            agg = [accp.tile([128, TPC], f32, tag=f"agg{f}", name=f"agg{f}")
                   for f in range(nF)]
            for bi, (t0, ntb, c0, bc, runs) in enumerate(blocks):
                qa = nc.sync if bi % 2 == 0 else nc.scalar
                qb = nc.scalar if bi % 2 == 0 else nc.sync
                h1 = (bc // 2) & ~7
                ew_t = sp.tile([128, BCMAX], bf, tag="ew")
                if variant == "reduceonly":
                    qa.dma_start(ew_t[:, 0:4], ew_d[bi][:, 0:4])
                elif mode == "layer2":
                    qa.dma_start(ew_t[:, 0:bc], ew_d[bi][:])
                else:
                    qa.dma_start(ew_t[:, 0:h1], ew_d[bi][:, 0:h1])
                    qb.dma_start(ew_t[:, h1:bc], ew_d[bi][:, h1:bc])
                if variant == "dmaonly":
                    continue
                if mode == "deg":
                    _reduce_2stage(ew_t, bc, runs, agg[0], "s1a")
                elif mode == "layer1":
                    xs_t = []
                    for f in range(D):
                        t_ = sp.tile([128, BCMAX], bf, tag=f"xs{f}")
                        if variant != "reduceonly":
                            qb.dma_start(t_[:, 0:h1], xs_d[f][bi][:, 0:h1])
                            qa.dma_start(t_[:, h1:bc], xs_d[f][bi][:, h1:bc])
                        else:
                            qb.dma_start(t_[:, 0:4], xs_d[f][bi][:, 0:4])
                        xs_t.append(t_)
                    m0 = wp.tile([128, BCMAX], bf, tag="m0")
                    nc.vector.tensor_mul(m0[:, 0:bc], ew_t[:, 0:bc],
                                         xs_t[0][:, 0:bc])
                    m1 = wp.tile([128, BCMAX], bf, tag="m1")
                    nc.vector.tensor_mul(m1[:, 0:bc], ew_t[:, 0:bc],
                                         xs_t[1][:, 0:bc])
                    _reduce_2stage(m0, bc, runs, agg[0], "s1a")
                    _reduce_2stage(m1, bc, runs, agg[1], "s1b")
                else:
                    vs_t = sp.tile([128, BCMAX], bf, tag="vs")
                    if variant != "reduceonly":
                        qb.dma_start(vs_t[:, 0:bc], vs_d[bi][:])
                    else:
                        qb.dma_start(vs_t[:, 0:4], vs_d[bi][:, 0:4])
                    m0 = wp.tile([128, BCMAX], bf, tag="m0")
                    nc.vector.tensor_mul(m0[:, 0:bc], ew_t[:, 0:bc],
                                         vs_t[:, 0:bc])
                    _reduce_2stage(m0, bc, runs, agg[0], "s1a")

            if variant in ("dmaonly", "reduceonly", "noepi"):
                return
            # ---- epilogue (self-loop slots make agg complete: deg sweep
            # yields deg+1; layer sweeps include the dinv*val self term) ----
            if mode == "deg":
                sq = wp.tile([128, TPC], f32, tag="sq")
                nc.scalar.activation(sq, agg[0],
                                     mybir.ActivationFunctionType.Sqrt)
                pk = wp.tile([128, (D + 1) * TPC], f32, tag="pk")
                nc.vector.reciprocal(pk[:, 0:TPC], sq)
                for f in range(D):
                    nc.vector.tensor_mul(pk[:, (1 + f) * TPC:(2 + f) * TPC],
                                         xN[f], pk[:, 0:TPC])
                nc.sync.dma_start(deg_out[:], pk[:])
            elif mode == "layer1":
                zb = []
                for f in range(D):
                    zb_ = wp.tile([128, TPC], bf, tag=f"zb{f}")
                    nc.vector.tensor_mul(zb_, agg[f], dinvN)
                    zb.append(zb_)
                # h layout [128, (j t)]: hidden-unit major; weights arrive
                # pre-materialized in the same layout so every product runs
                # in the 2x packed-bf16 DVE mode.
                hA = wp.tile([128, TPC * HID], bf, tag="hA")
                nc.vector.tensor_tensor(
                    hA.rearrange("p (j t) -> p j t", j=HID),
                    zb[0].unsqueeze(1).broadcast_to([128, HID, TPC]),
                    w1x[0].rearrange("p (j t) -> p j t", j=HID),
                    mybir.AluOpType.mult)
                hB = wp.tile([128, TPC * HID], bf, tag="hB")
                nc.vector.tensor_tensor(
                    hB.rearrange("p (j t) -> p j t", j=HID),
                    zb[1].unsqueeze(1).broadcast_to([128, HID, TPC]),
                    w1x[1].rearrange("p (j t) -> p j t", j=HID),
                    mybir.AluOpType.mult)
                nc.vector.tensor_add(hA, hA, hB)
                if not skip_b1:
                    nc.vector.tensor_tensor(
                        hA.rearrange("p (j t) -> p j t", j=HID),
                        hA.rearrange("p (j t) -> p j t", j=HID),
                        b1_sb.unsqueeze(2).broadcast_to([128, HID, TPC]),
                        mybir.AluOpType.add)
                if npos > 0:
                    nc.vector.tensor_scalar_max(hA[:, 0:npos * TPC],
                                                hA[:, 0:npos * TPC], 0.0)
                if npos < HID:
                    nc.vector.tensor_scalar_min(hA[:, npos * TPC:],
                                                hA[:, npos * TPC:], 0.0)
                T8 = 8 * TPC
                va = wp.tile([128, T8], bf, tag="va")
                nc.vector.tensor_add(va, hA[:, 0:T8], hA[:, T8:2 * T8])
                vb = wp.tile([128, T8 // 2], bf, tag="vb")
                nc.vector.tensor_add(vb, va[:, 0:T8 // 2], va[:, T8 // 2:T8])
                vc = wp.tile([128, T8 // 4], bf, tag="vc")
                nc.vector.tensor_add(vc, vb[:, 0:T8 // 4],
                                     vb[:, T8 // 4:T8 // 2])
                vd = wp.tile([128, TPC], f32, tag="vd")
                nc.vector.tensor_add(vd, vc[:, 0:TPC], vc[:, TPC:2 * TPC])
                vt = wp.tile([128, TPC], f32, tag="vt")
                nc.vector.tensor_mul(vt, vd, dinvN)
                nc.sync.dma_start(vt_out[:], vt[:])
            else:
                y = wp.tile([128, TPC], f32, tag="y")
                nc.vector.tensor_mul(y, agg[0], dinvN)
                if not skip_b2:
                    nc.vector.tensor_scalar(y, y, b2_sb[:, 0:1], None,
                                            mybir.AluOpType.add)
                nc.sync.dma_start(y_out[:], y[:])

        if reps > 1:
            assert reps % unroll == 0
            with tc.For_i(0, reps // unroll, 1):
                for _ in range(unroll):
                    body()
        else:
            body()

    _split_multi_waits(nc)
    return nc


def _rep_bf16(vec):
    return np.ascontiguousarray(
        np.tile(np.asarray(vec, np.float32).reshape(1, -1), (128, 1))
    ).astype(BF16)


def kernel(x, edge_index, edge_weight, W1, b1, W2, b2):
    x = np.asarray(x, np.float32)
    edge_index = np.asarray(edge_index)
    edge_weight = np.asarray(edge_weight, np.float32)
    W1 = np.asarray(W1, np.float32)
    b1 = np.asarray(b1, np.float32)
    W2 = np.asarray(W2, np.float32)
    b2 = np.asarray(b2, np.float32)
    skip_b1 = not np.any(b1 != 0)
    skip_b2 = not np.any(b2 != 0)

    pp = _preprocess(edge_index, edge_weight)
    order = pp["order"]

    ew_cs = _stream_blocks(pp, pp["ew"], "ew", BF16)

    xfull = np.zeros((NPAD, D), np.float32)
    xfull[:N] = x
    xnew = xfull[order]                          # newpos layout
    xn_cs = [_to_core_nodes(xnew[:, f], np.float32) for f in range(D)]

    # ---- NEFF 1: deg+1 -> dinv, x*dinv ----
    nc1 = _build_sweep("deg", pp)
    in1 = [dict(ew_cs[c], xn0=xn_cs[0][c], xn1=xn_cs[1][c])
           for c in range(NCORE)]
    r1 = run_bass_kernel_spmd(nc1, in1, core_ids=list(range(NCORE)))
    pk = [np.asarray(r1.results[c]["degout"], np.float32) for c in range(NCORE)]
    dinv_new = _from_core_nodes([p[:, 0:TPC] for p in pk])
    xt_new = [_from_core_nodes([p[:, (1 + f) * TPC:(2 + f) * TPC] for p in pk])
              for f in range(D)]

    # ---- host glue: per-edge (x*dinv)[src] streams ----
    xt_orig = np.empty((NPAD, D), np.float32)
    for f in range(D):
        xt_orig[order, f] = xt_new[f]
    xs_cs = [_stream_blocks(pp, xt_orig[pp["src"], f], f"xs{f}_", BF16)
             for f in range(D)]
    dinv_n = _to_core_nodes(dinv_new, np.float32)

    # fold W2 into W1 (g_j = h_j * w2_j); order non-negative-w2 units first
    w2v = W2[:, 0]
    perm = np.argsort(w2v < 0, kind="stable")
    npos = int((w2v >= 0).sum())
    W1p = (W1 * w2v[None, :])[:, perm]
    b1p = (b1 * w2v)[perm]
    w1x = [_rep_bf16(np.repeat(W1p[f], TPC)) for f in range(D)]
    b1b = _rep_bf16(b1p)
    b2b = np.full((128, 1), float(b2[0]), np.float32)

    # ---- NEFF 2: layer 1 -> v*dinv ----
    nc2 = _build_sweep("layer1", pp, skip_b1=skip_b1, skip_b2=skip_b2,
                       npos=npos)
    in2 = [dict(ew_cs[c], **xs_cs[0][c], **xs_cs[1][c],
                dinvn=dinv_n[c], w1x0b=w1x[0], w1x1b=w1x[1], b1b=b1b)
           for c in range(NCORE)]
    r2 = run_bass_kernel_spmd(nc2, in2, core_ids=list(range(NCORE)))
    vt_new = _from_core_nodes([r2.results[c]["vtout"] for c in range(NCORE)])

    # ---- host glue: (v*dinv)[src] stream ----
    vt_orig = np.empty(NPAD, np.float32)
    vt_orig[order] = vt_new
    vs_cs = _stream_blocks(pp, vt_orig[pp["src"]], "vs", BF16)
    dinv_n2 = dinv_n

    # ---- NEFF 3: layer 2 -> output ----
    nc3 = _build_sweep("layer2", pp, skip_b1=skip_b1, skip_b2=skip_b2)
    in3 = [dict(ew_cs[c], **vs_cs[c], dinvn=dinv_n2[c], b2b=b2b)
           for c in range(NCORE)]
    r3 = run_bass_kernel_spmd(nc3, in3, core_ids=list(range(NCORE)))
    y_new = _from_core_nodes([r3.results[c]["yout"] for c in range(NCORE)])

    y_orig = np.empty(NPAD, np.float32)
    y_orig[order] = y_new
    return y_orig[:N, None].astype(np.float32)


# revision 26
# speedup vs baseline: 1.0864x; 1.0864x over previous
"""GCN (2-layer, PyG gcn_norm) on 8 Trainium2 NeuronCores via Bass.

Strategy (dst-partition-row sharding, no collectives, no PE):
  * Host appends self-loop edges (weight 1, as in gcn_norm), sorts nodes
    by in-degree and assigns each node one SBUF partition-row of G slots
    (G = per-stripe max degree rounded up to 8; ~12% padding), so the
    per-node segment-sum needs no one-hot masks or matmuls.  Stripes of
    1024 nodes (one 128-node tile per core) share a G schedule so all 8
    SPMD cores run one program.
  * Per-edge streams are bf16 (tolerance is 2e-2).  The segment-sum runs
    as a packed-bf16 pair-add tree on DVE (tensor_tensor at the 2x rate;
    TensorReduce itself has no fast mode) with a final narrow f32
    tensor_reduce per equal-G run.  The layer-1 node epilogue
    (z->W1->relu->W2) uses weight tiles pre-materialized in (hidden,
    tile) layout so every product is a packed 2x tensor_tensor.
  * Streams transfer as whole contiguous blocks split across the two
    hardware DGE queues (SP + Activation); the Activation engine is kept
    compute-free so its queue can prefetch ahead of the epilogue.
  * Three sequential NEFF launches: (1) deg -> dinv, x*dinv, (2) layer-1
    aggregation -> h -> v*dinv, (3) layer-2 aggregation -> output.
    Between launches the host only gathers returned per-node arrays into
    per-edge streams (index-space data movement, no float math).
"""

import sys

sys.path.insert(0, "/opt/trn_rl_repo")

import numpy as np
import ml_dtypes

import concourse.bass as bass
import concourse.tile as tile
from concourse import mybir
from concourse.bass_utils import run_bass_kernel_spmd

BF16 = ml_dtypes.bfloat16

N = 100000
E = 3200000
D = 2
HID = 16
NCORE = 8
TPC = 98                      # stripes == node tiles per core
NPAD = TPC * 1024             # 100352
GMULT = 8                     # stripe slot width rounded up to this
BLK_COLS = 4096               # target stream columns per DMA block (>= CS: single block)


def _split_multi_waits(nc):
    """This toolchain's walrus encodes at most one sync-wait per instruction.
    Hoist extra waits onto fresh single-wait NoOps placed just before."""
    ctr = 0
    for fn in nc.m.functions:
        for bb in fn.blocks:
            insts = list(bb.instructions)
            if not any(
                i.sync_info is not None and len(i.sync_info.on_wait or []) > 1
                for i in insts
            ):
                continue
            new = []
            for inst in insts:
                si = inst.sync_info
                if si is not None and len(si.on_wait or []) > 1:
                    waits = list(si.on_wait)
                    for w in waits[:-1]:
                        ctr += 1
                        new.append(
                            mybir.InstNoOp(
                                name=f"wsplit-{ctr}",
                                engine=inst.engine,
                                sync_info=mybir.SyncInfo(on_wait=[w], on_update=[]),
                                bass_nofuse=True,
                            )
                        )
                    si.on_wait = [waits[-1]]
                new.append(inst)
            bb.instructions = new
    return ctr


def _preprocess(edge_index, edge_weight):
    """Append self-loops, degree-sort nodes, assign each node a
    partition-row slot range, and scatter edge weight / src index into the
    per-core slot streams."""
    loop = np.arange(N, dtype=np.int64)
    dst = np.concatenate([edge_index[1].astype(np.int64), loop])
    src = np.concatenate([edge_index[0].astype(np.int64), loop])
    ew = np.concatenate([edge_weight.astype(np.float32),
                         np.ones(N, np.float32)])
    ne = len(dst)

    deg = np.bincount(dst, minlength=NPAD)
    order = np.argsort(deg, kind="stable")       # newpos -> orig id
    newpos = np.empty(NPAD, np.int64)
    newpos[order] = np.arange(NPAD)

    counts_new = deg[order]                      # per-newpos degree
    smax = counts_new.reshape(TPC, 1024).max(axis=1)
    G = np.maximum(GMULT, ((smax + GMULT - 1) // GMULT) * GMULT).astype(np.int64)
    offs = np.zeros(TPC + 1, np.int64)
    np.cumsum(G, out=offs[1:])
    CS = int(offs[-1])

    nd = newpos[dst]
    start = np.zeros(NPAD + 1, np.int64)
    np.cumsum(counts_new, out=start[1:])
    perm = np.argsort(nd, kind="stable")
    r = np.empty(ne, np.int64)
    r[perm] = np.arange(ne) - start[nd[perm]]    # rank of edge within its dst

    s = nd >> 10
    w = nd & 1023
    c = w >> 7
    p = w & 127
    flat = (c * 128 + p) * CS + offs[s] + r

    ew_flat = np.zeros(NCORE * 128 * CS, np.float32)
    src_flat = np.zeros(NCORE * 128 * CS, np.int64)
    ew_flat[flat] = ew
    src_flat[flat] = src

    # DMA blocks: consecutive stripes until >= BLK_COLS columns; per-block
    # runs of stripes sharing G (one tensor_reduce instruction per run).
    blocks = []
    t0, cols = 0, 0
    for t in range(TPC):
        cols += int(G[t])
        if cols >= BLK_COLS or t == TPC - 1:
            runs = []
            ro = 0
            for tt in range(t0, t + 1):
                g = int(G[tt])
                if runs and runs[-1][2] == g:
                    runs[-1] = (runs[-1][0], runs[-1][1] + 1, g, runs[-1][3])
                else:
                    runs.append((tt, 1, g, ro))
                ro += g
            blocks.append((t0, t + 1 - t0, int(offs[t0]), cols, runs))
            t0, cols = t + 1, 0

    return dict(G=G, offs=offs, CS=CS, blocks=blocks, order=order,
                ew=ew_flat, src=src_flat)


def _stream_blocks(sched, arrflat, prefix, dtype):
    """Per-core dicts of per-DMA-block contiguous stream arrays."""
    CS = sched["CS"]
    a = arrflat.reshape(NCORE, 128, CS)
    out = []
    for c in range(NCORE):
        d = {}
        for bi, (t0, ntb, c0, bc, runs) in enumerate(sched["blocks"]):
            d[f"{prefix}{bi}"] = np.ascontiguousarray(
                a[c, :, c0:c0 + bc]).astype(dtype)
        out.append(d)
    return out


def _to_core_nodes(val_new, dtype):
    """[NPAD] array in newpos space -> per-core [128, TPC]
    (newpos = s*1024 + c*128 + p)."""
    a = val_new.reshape(TPC, NCORE, 128)
    return [np.ascontiguousarray(a[:, c, :].T).astype(dtype) for c in range(NCORE)]


def _from_core_nodes(parts):
    full = np.empty((TPC, NCORE, 128), np.float32)
    for c in range(NCORE):
        full[:, c, :] = np.asarray(parts[c], np.float32).T
    return full.reshape(NPAD)


def _build_sweep(mode, sched, reps=1, variant=None, unroll=16,
                 skip_b1=True, skip_b2=True, npos=HID):
    """Build the Bass program for one sweep. mode in {deg, layer1, layer2}.
    reps>1 wraps `reps` copies of the (idempotent) body in a hardware For_i
    loop, `unroll` bodies per trip — used only for timing measurements.
    variant (timing experiments only): 'dmaonly' = stream DMA without
    compute, 'reduceonly' = compute without stream DMA."""
    from contextlib import ExitStack

    CS = sched["CS"]
    blocks = sched["blocks"]
    BCMAX = max(b[3] for b in blocks)
    f32 = mybir.dt.float32
    bf = mybir.dt.bfloat16

    nc = bass.Bass("TRN2", target_bir_lowering=False, debug=False,
                   num_devices=NCORE)

    def din(name, shape, dtype=f32):
        return nc.dram_tensor(name, shape, dtype, kind="ExternalInput").ap()

    def dout(name, shape, dtype=f32):
        return nc.dram_tensor(name, shape, dtype, kind="ExternalOutput").ap()

    def din_blocks(prefix):
        return [din(f"{prefix}{bi}", [128, b[3]], bf)
                for bi, b in enumerate(blocks)]

    ew_d = din_blocks("ew")
    if mode == "deg":
        xn_d = [din(f"xn{f}", [128, TPC]) for f in range(D)]
        deg_out = dout("degout", [128, (D + 1) * TPC])
    elif mode == "layer1":
        xs_d = [din_blocks(f"xs{f}_") for f in range(D)]
        dinv_d = din("dinvn", [128, TPC])
        w1x_d = [din(f"w1x{f}b", [128, HID * TPC], bf) for f in range(D)]
        b1_d = din("b1b", [128, HID], bf)
        vt_out = dout("vtout", [128, TPC])
    else:
        vs_d = din_blocks("vs")
        dinv_d = din("dinvn", [128, TPC])
        b2_d = din("b2b", [128, 1])
        y_out = dout("yout", [128, TPC])

    with tile.TileContext(nc) as tc, ExitStack() as ctx:
        nbuf = 3 if mode == "layer1" else 6
        const = ctx.enter_context(tc.tile_pool(name="const", bufs=1))
        sp = ctx.enter_context(tc.tile_pool(name="streams", bufs=nbuf))
        wp = ctx.enter_context(tc.tile_pool(name="work", bufs=nbuf))
        accp = ctx.enter_context(tc.tile_pool(name="acc", bufs=2))

        if mode == "deg":
            xN = []
            for f in range(D):
                t_ = const.tile([128, TPC], f32, tag=f"xn{f}")
                nc.sync.dma_start(t_[:], xn_d[f][:])
                xN.append(t_)
        elif mode == "layer1":
            dinvN = const.tile([128, TPC], f32)
            nc.sync.dma_start(dinvN[:], dinv_d[:])
            w1x = []
            for f in range(D):
                t_ = const.tile([128, HID * TPC], bf, tag=f"w1x{f}")
                nc.sync.dma_start(t_[:], w1x_d[f][:])
                w1x.append(t_)
            b1_sb = const.tile([128, HID], bf)
            nc.sync.dma_start(b1_sb[:], b1_d[:])
        else:
            dinvN = const.tile([128, TPC], f32)
            nc.sync.dma_start(dinvN[:], dinv_d[:])
            b2_sb = const.tile([128, 1], f32)
            nc.sync.dma_start(b2_sb[:], b2_d[:])

        nF = D if mode == "layer1" else 1

        def _reduce_2stage(m_t, bc, runs, dst_agg, s1tag):
            # Pair-add tree: TensorReduce has no fast DVE mode (1x), but
            # TensorTensor adds on packed bf16 sub-slices run at 2x.  Two
            # tree levels collapse each 8-slot group to 2 partials; a final
            # narrow f32 TensorReduce finishes per G-run.
            u = wp.tile([128, BCMAX // 2], bf, tag=s1tag + "u")
            mv = m_t[:, 0:bc].rearrange("p (q g) -> p q g", g=8)
            nc.vector.tensor_tensor(
                u[:, 0:bc // 2].rearrange("p (q h) -> p q h", h=4),
                mv[:, :, 0:4], mv[:, :, 4:8], mybir.AluOpType.add)
            w = wp.tile([128, BCMAX // 4], bf, tag=s1tag + "w")
            uv = u[:, 0:bc // 2].rearrange("p (q h) -> p q h", h=4)
            nc.vector.tensor_tensor(
                w[:, 0:bc // 4].rearrange("p (q h) -> p q h", h=2),
                uv[:, :, 0:2], uv[:, :, 2:4], mybir.AluOpType.add)
            for (tt, nt, g, ro) in runs:
                nc.vector.tensor_reduce(
                    dst_agg[:, tt:tt + nt],
                    w[:, ro // 4:ro // 4 + nt * (g // 4)].rearrange(
                        "p (t q) -> p t q", q=g // 4),
                    mybir.AxisListType.X, mybir.AluOpType.add)

        def body():
            agg = [accp.tile([128, TPC], f32, tag=f"agg{f}", name=f"agg{f}")
                   for f in range(nF)]
            for bi, (t0, ntb, c0, bc, runs) in enumerate(blocks):
                qa = nc.sync if bi % 2 == 0 else nc.scalar
                qb = nc.scalar if bi % 2 == 0 else nc.sync
                h1 = (bc // 2) & ~7
                ew_t = sp.tile([128, BCMAX], bf, tag="ew")
                if variant == "reduceonly":
                    qa.dma_start(ew_t[:, 0:4], ew_d[bi][:, 0:4])
                elif mode == "layer2":
                    qa.dma_start(ew_t[:, 0:bc], ew_d[bi][:])
                else:
                    qa.dma_start(ew_t[:, 0:h1], ew_d[bi][:, 0:h1])
                    qb.dma_start(ew_t[:, h1:bc], ew_d[bi][:, h1:bc])
                if variant == "dmaonly":
                    continue
                if mode == "deg":
                    _reduce_2stage(ew_t, bc, runs, agg[0], "s1a")
                elif mode == "layer1":
                    xs_t = []
                    for f in range(D):
                        t_ = sp.tile([128, BCMAX], bf, tag=f"xs{f}")
                        if variant != "reduceonly":
                            qb.dma_start(t_[:, 0:h1], xs_d[f][bi][:, 0:h1])
                            qa.dma_start(t_[:, h1:bc], xs_d[f][bi][:, h1:bc])
                        else:
                            qb.dma_start(t_[:, 0:4], xs_d[f][bi][:, 0:4])
                        xs_t.append(t_)
                    if bc == BCMAX:
                        mm = wp.tile([128, 2 * BCMAX], bf, tag="mm")
                        nc.vector.tensor_mul(mm[:, 0:bc], ew_t[:, 0:bc],
                                             xs_t[0][:, 0:bc])
                        nc.vector.tensor_mul(mm[:, BCMAX:BCMAX + bc],
                                             ew_t[:, 0:bc], xs_t[1][:, 0:bc])
                        uu = wp.tile([128, BCMAX], bf, tag="uu")
                        mv = mm.rearrange("p (q g) -> p q g", g=8)
                        nc.vector.tensor_tensor(
                            uu.rearrange("p (q h) -> p q h", h=4),
                            mv[:, :, 0:4], mv[:, :, 4:8], mybir.AluOpType.add)
                        ww = wp.tile([128, BCMAX // 2], bf, tag="ww")
                        uv = uu.rearrange("p (q h) -> p q h", h=4)
                        nc.vector.tensor_tensor(
                            ww.rearrange("p (q h) -> p q h", h=2),
                            uv[:, :, 0:2], uv[:, :, 2:4], mybir.AluOpType.add)
                        for f in range(D):
                            w_f = ww[:, f * (BCMAX // 4):(f + 1) * (BCMAX // 4)]
                            for (tt, nt, g, ro) in runs:
                                nc.vector.tensor_reduce(
                                    agg[f][:, tt:tt + nt],
                                    w_f[:, ro // 4:ro // 4 + nt * (g // 4)]
                                    .rearrange("p (t q) -> p t q", q=g // 4),
                                    mybir.AxisListType.X, mybir.AluOpType.add)
                    else:
                        m0 = wp.tile([128, BCMAX], bf, tag="m0")
                        nc.vector.tensor_mul(m0[:, 0:bc], ew_t[:, 0:bc],
                                             xs_t[0][:, 0:bc])
                        m1 = wp.tile([128, BCMAX], bf, tag="m1")
                        nc.vector.tensor_mul(m1[:, 0:bc], ew_t[:, 0:bc],
                                             xs_t[1][:, 0:bc])
                        _reduce_2stage(m0, bc, runs, agg[0], "s1a")
                        _reduce_2stage(m1, bc, runs, agg[1], "s1b")
                else:
                    vs_t = sp.tile([128, BCMAX], bf, tag="vs")
                    if variant != "reduceonly":
                        qb.dma_start(vs_t[:, 0:bc], vs_d[bi][:])
                    else:
                        qb.dma_start(vs_t[:, 0:4], vs_d[bi][:, 0:4])
                    m0 = wp.tile([128, BCMAX], bf, tag="m0")
                    nc.vector.tensor_mul(m0[:, 0:bc], ew_t[:, 0:bc],
                                         vs_t[:, 0:bc])
                    _reduce_2stage(m0, bc, runs, agg[0], "s1a")

            if variant in ("dmaonly", "reduceonly", "noepi"):
                return
            # ---- epilogue (self-loop slots make agg complete: deg sweep
            # yields deg+1; layer sweeps include the dinv*val self term) ----
            if mode == "deg":
                sq = wp.tile([128, TPC], f32, tag="sq")
                nc.scalar.activation(sq, agg[0],
                                     mybir.ActivationFunctionType.Sqrt)
                pk = wp.tile([128, (D + 1) * TPC], f32, tag="pk")
                nc.vector.reciprocal(pk[:, 0:TPC], sq)
                for f in range(D):
                    nc.vector.tensor_mul(pk[:, (1 + f) * TPC:(2 + f) * TPC],
                                         xN[f], pk[:, 0:TPC])
                nc.sync.dma_start(deg_out[:], pk[:])
            elif mode == "layer1":
                zb = []
                for f in range(D):
                    zb_ = wp.tile([128, TPC], bf, tag=f"zb{f}")
                    nc.vector.tensor_mul(zb_, agg[f], dinvN)
                    zb.append(zb_)
                # h layout [128, (j t)]: hidden-unit major; weights arrive
                # pre-materialized in the same layout so every product runs
                # in the 2x packed-bf16 DVE mode.
                hA = wp.tile([128, TPC * HID], bf, tag="hA")
                nc.vector.tensor_tensor(
                    hA.rearrange("p (j t) -> p j t", j=HID),
                    zb[0].unsqueeze(1).broadcast_to([128, HID, TPC]),
                    w1x[0].rearrange("p (j t) -> p j t", j=HID),
                    mybir.AluOpType.mult)
                hB = wp.tile([128, TPC * HID], bf, tag="hB")
                nc.vector.tensor_tensor(
                    hB.rearrange("p (j t) -> p j t", j=HID),
                    zb[1].unsqueeze(1).broadcast_to([128, HID, TPC]),
                    w1x[1].rearrange("p (j t) -> p j t", j=HID),
                    mybir.AluOpType.mult)
                nc.vector.tensor_add(hA, hA, hB)
                if not skip_b1:
                    nc.vector.tensor_tensor(
                        hA.rearrange("p (j t) -> p j t", j=HID),
                        hA.rearrange("p (j t) -> p j t", j=HID),
                        b1_sb.unsqueeze(2).broadcast_to([128, HID, TPC]),
                        mybir.AluOpType.add)
                if npos > 0:
                    nc.vector.tensor_scalar_max(hA[:, 0:npos * TPC],
                                                hA[:, 0:npos * TPC], 0.0)
                if npos < HID:
                    nc.vector.tensor_scalar_min(hA[:, npos * TPC:],
                                                hA[:, npos * TPC:], 0.0)
                T8 = 8 * TPC
                va = wp.tile([128, T8], bf, tag="va")
                nc.vector.tensor_add(va, hA[:, 0:T8], hA[:, T8:2 * T8])
                vb = wp.tile([128, T8 // 2], bf, tag="vb")
                nc.vector.tensor_add(vb, va[:, 0:T8 // 2], va[:, T8 // 2:T8])
                vc = wp.tile([128, T8 // 4], bf, tag="vc")
                nc.vector.tensor_add(vc, vb[:, 0:T8 // 4],
                                     vb[:, T8 // 4:T8 // 2])
                vd = wp.tile([128, TPC], f32, tag="vd")
                nc.vector.tensor_add(vd, vc[:, 0:TPC], vc[:, TPC:2 * TPC])
                vt = wp.tile([128, TPC], f32, tag="vt")
                nc.vector.tensor_mul(vt, vd, dinvN)
                nc.sync.dma_start(vt_out[:], vt[:])
            else:
                y = wp.tile([128, TPC], f32, tag="y")
                nc.vector.tensor_mul(y, agg[0], dinvN)
                if not skip_b2:
                    nc.vector.tensor_scalar(y, y, b2_sb[:, 0:1], None,
                                            mybir.AluOpType.add)
                nc.sync.dma_start(y_out[:], y[:])

        if reps > 1:
            assert reps % unroll == 0
            with tc.For_i(0, reps // unroll, 1):
                for _ in range(unroll):
                    body()
        else:
            body()

    _split_multi_waits(nc)
    return nc


def _rep_bf16(vec):
    return np.ascontiguousarray(
        np.tile(np.asarray(vec, np.float32).reshape(1, -1), (128, 1))
    ).astype(BF16)


def kernel(x, edge_index, edge_weight, W1, b1, W2, b2):
    x = np.asarray(x, np.float32)
    edge_index = np.asarray(edge_index)
    edge_weight = np.asarray(edge_weight, np.float32)
    W1 = np.asarray(W1, np.float32)
    b1 = np.asarray(b1, np.float32)
    W2 = np.asarray(W2, np.float32)
    b2 = np.asarray(b2, np.float32)
    skip_b1 = not np.any(b1 != 0)
    skip_b2 = not np.any(b2 != 0)

    pp = _preprocess(edge_index, edge_weight)
    order = pp["order"]

    ew_cs = _stream_blocks(pp, pp["ew"], "ew", BF16)

    xfull = np.zeros((NPAD, D), np.float32)
    xfull[:N] = x
    xnew = xfull[order]                          # newpos layout
    xn_cs = [_to_core_nodes(xnew[:, f], np.float32) for f in range(D)]

    # ---- NEFF 1: deg+1 -> dinv, x*dinv ----
    nc1 = _build_sweep("deg", pp)
    in1 = [dict(ew_cs[c], xn0=xn_cs[0][c], xn1=xn_cs[1][c])
           for c in range(NCORE)]
    r1 = run_bass_kernel_spmd(nc1, in1, core_ids=list(range(NCORE)))
    pk = [np.asarray(r1.results[c]["degout"], np.float32) for c in range(NCORE)]
    dinv_new = _from_core_nodes([p[:, 0:TPC] for p in pk])
    xt_new = [_from_core_nodes([p[:, (1 + f) * TPC:(2 + f) * TPC] for p in pk])
              for f in range(D)]

    # ---- host glue: per-edge (x*dinv)[src] streams ----
    xt_orig = np.empty((NPAD, D), np.float32)
    for f in range(D):
        xt_orig[order, f] = xt_new[f]
    xs_cs = [_stream_blocks(pp, xt_orig[pp["src"], f], f"xs{f}_", BF16)
             for f in range(D)]
    dinv_n = _to_core_nodes(dinv_new, np.float32)

    # fold W2 into W1 (g_j = h_j * w2_j); order non-negative-w2 units first
    w2v = W2[:, 0]
    perm = np.argsort(w2v < 0, kind="stable")
    npos = int((w2v >= 0).sum())
    W1p = (W1 * w2v[None, :])[:, perm]
    b1p = (b1 * w2v)[perm]
    w1x = [_rep_bf16(np.repeat(W1p[f], TPC)) for f in range(D)]
    b1b = _rep_bf16(b1p)
    b2b = np.full((128, 1), float(b2[0]), np.float32)

    # ---- NEFF 2: layer 1 -> v*dinv ----
    nc2 = _build_sweep("layer1", pp, skip_b1=skip_b1, skip_b2=skip_b2,
                       npos=npos)
    in2 = [dict(ew_cs[c], **xs_cs[0][c], **xs_cs[1][c],
                dinvn=dinv_n[c], w1x0b=w1x[0], w1x1b=w1x[1], b1b=b1b)
           for c in range(NCORE)]
    r2 = run_bass_kernel_spmd(nc2, in2, core_ids=list(range(NCORE)))
    vt_new = _from_core_nodes([r2.results[c]["vtout"] for c in range(NCORE)])

    # ---- host glue: (v*dinv)[src] stream ----
    vt_orig = np.empty(NPAD, np.float32)
    vt_orig[order] = vt_new
    vs_cs = _stream_blocks(pp, vt_orig[pp["src"]], "vs", BF16)
    dinv_n2 = dinv_n

    # ---- NEFF 3: layer 2 -> output ----
    nc3 = _build_sweep("layer2", pp, skip_b1=skip_b1, skip_b2=skip_b2)
    in3 = [dict(ew_cs[c], **vs_cs[c], dinvn=dinv_n2[c], b2b=b2b)
           for c in range(NCORE)]
    r3 = run_bass_kernel_spmd(nc3, in3, core_ids=list(range(NCORE)))
    y_new = _from_core_nodes([r3.results[c]["yout"] for c in range(NCORE)])

    y_orig = np.empty(NPAD, np.float32)
    y_orig[order] = y_new
    return y_orig[:N, None].astype(np.float32)
